# revision 40
# baseline (speedup 1.0000x reference)
"""Trainium2 Bass kernel for nn_MixedPredictor (gnn_message_passing).

final[e] = softmax(gates)[0] * dot(h_user[src[e]], h_item[dst[e]])
         + softmax(gates)[1] * MLP(concat(h_user[src[e]], h_item[dst[e]]))

Strategy (8 NeuronCores, data-parallel over edges):
  - Edge rows are fetched with InstDMAGatherAnt (nc.gpsimd.dma_gather):
    one instruction gathers 1024-2048 rows (994 ns fixed + 0.34 ns/desc on
    the Pool engine) vs indirect_dma_start's 128 rows per ~1 us call. Node
    tables are repacked host-side into 512-byte mixed rows
    [bf16 f0..f127 | fp8e4m3 pair-packed f0..f127 | pad] so transpose=True
    lands each gather FEATURE-MAJOR in SBUF (no PE transposes, no
    PSUM->SBUF copies) with both a bf16 plane (dot product, gate layer) and
    an fp8 plane (MLP layers 1-2 run as DoubleRow matmuls at 2x PE rate).
    512 B descriptors ride the DMA latency floor, so the fp8 plane is free.
  - dma_gather indices are int16, so the 100k-row tables are split in 4
    ranges of 25000 rows. Edges are bucketed host-side by (src_range,
    dst_range); each 4096-edge macro serves one PAIR of dst ranges
    (even macros dr 0/1, odd macros dr 2/3) with a 512-slot quota per
    bucket, giving 4 src gathers (1024 idxs) + 2 dst gathers (2048) per
    macro, each writing one contiguous span.
  - Layer 3 and the gate layer share one PSUM tile ([0:64)/[64:128)
    partitions) -> one relu covers both. The heads (mlp, gate-delta, dot)
    are three rows of one PSUM tile via two accumulating matmuls, PE-
    transposed to edge-major; the final combine
    final = dot + sigmoid(gd)*(mlp + b4 - dot) is batched once per macro.
  - Elementwise work is split so each engine's in-order queue only holds
    early-chain ops: Activation does the L1/L2 relus + sigmoid, DVE does the
    L3/gate relu, dot multiply, head copies and the batched tail. Gathers
    prefetch 3 macros ahead, so Pool descriptor generation never stalls
    compute.
"""

import numpy as np
import ml_dtypes

import concourse.bass as bass
import concourse.bacc as bacc
import concourse.mybir as mybir
import concourse.tile as tile
from concourse.bass_utils import run_bass_kernel_spmd

N_CORES = 8
N_USERS = 100000
N_ITEMS = 100000
N_EDGES = 500000
D = 128

MACRO = 4096            # edges per macro tile
NMACRO = 16             # macros per core (even: dr 0/1, odd: dr 2/3)
GROUPS = 8              # 512-edge groups per macro
NMAC_TOT = N_CORES * NMACRO      # 128
E_CORE = NMACRO * MACRO          # 65536
E_PAD = N_CORES * E_CORE         # 524288

RANGE = 25000           # rows per gather-index range (int16-safe)
NR = 4                  # index ranges covering 100k rows
QUOTA = 512             # slots per (sr, dr) bucket per eligible macro
ROW = 256               # mixed-row u16 elements: 128 bf16 + 64 fp8-pair + 64 pad

F32 = mybir.dt.float32
BF16 = mybir.dt.bfloat16
F8 = mybir.dt.float8e4
I16 = mybir.dt.int16
AF = mybir.ActivationFunctionType
ALU = mybir.AluOpType
DR = mybir.MatmulPerfMode.DoubleRow
BF = ml_dtypes.bfloat16
NF8 = ml_dtypes.float8_e4m3

_OFFS = [
    ("w1sf", 512), ("w1df", 512), ("w2f", 256), ("w3", 128),
    ("wg1s", 128), ("wg1d", 128), ("hw3", 8), ("ones3", 8),
    ("b1a", 4), ("b1b", 4), ("b2t", 4), ("b3g", 4), ("b4t", 4),
    ("bg2dt", 4), ("id3", 12), ("zeros", 1024),
]
OFF = {}
_o = 0
for _n, _sz in _OFFS:
    OFF[_n] = _o
    _o += _sz
CBYTES = _o

_CACHE = {}


def build_nc(nmacro=NMACRO):
    nc = bacc.Bacc(
        "TRN2",
        target_bir_lowering=False,
        debug=False,
        enable_asserts=False,
        num_devices=N_CORES,
    )

    hu = nc.dram_tensor("hum", [N_USERS, ROW], BF16, kind="ExternalInput").ap()
    hi = nc.dram_tensor("him", [N_ITEMS, ROW], BF16, kind="ExternalInput").ap()
    srcs = nc.dram_tensor("srcc", [NMACRO, 128, 256], I16, kind="ExternalInput").ap()
    dsts = nc.dram_tensor("dstc", [NMACRO, 128, 256], I16, kind="ExternalInput").ap()
    U8 = mybir.dt.uint8
    cbd = nc.dram_tensor("cblob", [128, CBYTES], U8, kind="ExternalInput").ap()

    out = nc.dram_tensor("out", [nmacro * MACRO], F32, kind="ExternalOutput").ap()

    with tile.TileContext(nc) as tc:
        with (
            tc.tile_pool(name="const", bufs=1) as cp,
            tc.tile_pool(name="gather", bufs=3) as gp,
            tc.tile_pool(name="work", bufs=4) as wp,
            tc.tile_pool(name="psum1", bufs=1, space="PSUM") as pp1,
            tc.tile_pool(name="psum2", bufs=1, space="PSUM") as pp2,
        ):
            # ---- constants: one packed byte-blob DMA, bitcast views ----
            cb = cp.tile([128, CBYTES], mybir.dt.uint8, tag="cb")
            nc.sync.dma_start(out=cb[:], in_=cbd[:, :])

            def cview(off, nbytes, dt_):
                return cb[:, off : off + nbytes].bitcast(dt_)

            w1sf = cview(OFF["w1sf"], 512, F8).rearrange("p (two m) -> p two m", two=2)
            w1df = cview(OFF["w1df"], 512, F8).rearrange("p (two m) -> p two m", two=2)
            w2f = cview(OFF["w2f"], 256, F8).rearrange("p (two m) -> p two m", two=2)
            w3t = cview(OFF["w3"], 128, BF16)
            wg1s = cview(OFF["wg1s"], 128, BF16)
            wg1dt = cview(OFF["wg1d"], 128, BF16)
            hw3 = cview(OFF["hw3"], 8, BF16)[:, 0:3]
            ones3 = cview(OFF["ones3"], 8, BF16)[:, 0:3]
            b1a = cview(OFF["b1a"], 4, F32)
            b1b = cview(OFF["b1b"], 4, F32)
            b2t = cview(OFF["b2t"], 4, F32)
            b3g = cview(OFF["b3g"], 4, F32)
            b4t = cview(OFF["b4t"], 4, F32)
            bg2dt = cview(OFF["bg2dt"], 4, F32)
            id3 = cb[0:3, OFF["id3"] : OFF["id3"] + 12].bitcast(F32)
            zeros = cview(OFF["zeros"], 1024, BF16)

            for m in range(nmacro):
                base = m * MACRO
                drb = 0 if m % 2 == 0 else 2   # dst ranges served by this macro

                idx_s = gp.tile([128, 256], I16, tag="idx_s")
                nc.sync.dma_start(out=idx_s[:], in_=srcs[m, :, :])
                idx_d = gp.tile([128, 256], I16, tag="idx_d")
                nc.sync.dma_start(out=idx_d[:], in_=dsts[m, :, :])

                # sg4[p, r, h, e]: src range r, u16-half h (0=bf16, 1=fp8), col e
                sg4 = gp.tile([128, NR, 2, 1024], BF16, tag="sg4")
                dg4 = gp.tile([128, 2, 2, 2048], BF16, tag="dg4")
                def sgather(r):
                    nc.gpsimd.dma_gather(
                        out_ap=sg4[:, r, :, :],
                        in_ap=hu[RANGE * r : RANGE * (r + 1), :],
                        idxs_ap=idx_s[:, 64 * r : 64 * (r + 1)],
                        num_idxs=1024,
                        num_idxs_reg=1024,
                        elem_size=ROW,
                        transpose=True,
                        single_packet=False,
                    )
                def dgather(q):
                    nc.gpsimd.dma_gather(
                        out_ap=dg4[:, q, :, :],
                        in_ap=hi[RANGE * (drb + q) : RANGE * (drb + q + 1), :],
                        idxs_ap=idx_d[:, 128 * q : 128 * (q + 1)],
                        num_idxs=2048,
                        num_idxs_reg=2048,
                        elem_size=ROW,
                        transpose=True,
                        single_packet=False,
                    )
                # src range 0 + both dst ranges first: groups 0-1 unblock
                # after 3 calls instead of 6
                sgather(0)
                dgather(0)
                dgather(1)
                for r in range(1, NR):
                    sgather(r)

                sg_f8 = sg4[:].bitcast(F8).rearrange(
                    "p r h (e two) -> p r h two e", two=2
                )
                dg_f8 = dg4[:].bitcast(F8).rearrange(
                    "p q h (e two) -> p q h two e", two=2
                )

                tt_all = pp1.tile([128, 12 * GROUPS], F32, tag="tt")

                for gpair in range(GROUPS // 2):
                  # ---- L1 (fp8 DoubleRow) for a PAIR of groups into one
                  # [128, 1024] PSUM so the relus run 1024 wide (halved init)
                  h1a2 = pp2.tile([128, 1024], F32, tag="h1a2")
                  h1b2 = pp2.tile([128, 1024], F32, tag="h1b2")
                  for g in (2 * gpair, 2 * gpair + 1):
                    sr, q = g // 2, g % 2
                    c0 = 512 * q
                    xs_f8 = sg_f8[:, sr, 1, :, c0 : c0 + 512]
                    xd_f8 = dg_f8[:, q, 1, :, 512 * sr : 512 * sr + 512]
                    csl = slice(512 * (g % 2), 512 * (g % 2) + 512)
                    for msl, h1 in ((slice(0, 128), h1a2), (slice(128, 256), h1b2)):
                        nc.tensor.matmul(
                            out=h1[:, csl], lhsT=w1sf[:, :, msl], rhs=xs_f8,
                            perf_mode=DR, start=True, stop=False,
                        )
                        nc.tensor.matmul(
                            out=h1[:, csl], lhsT=w1df[:, :, msl], rhs=xd_f8,
                            perf_mode=DR, start=False, stop=True,
                        )
                  # h1s4[p, pair, plane, e]: relu'd h1 in fp8, both groups
                  h1s4 = wp.tile([128, 2, 2, 512], F8, tag="h1s4")
                  nc.scalar.activation(
                      out=h1s4[:, :, 0, :], in_=h1a2[:], func=AF.Relu, bias=b1a[:],
                  )
                  nc.scalar.activation(
                      out=h1s4[:, :, 1, :], in_=h1b2[:], func=AF.Relu, bias=b1b[:],
                  )

                  for g in (2 * gpair, 2 * gpair + 1):
                    sr, q = g // 2, g % 2
                    c0 = 512 * q
                    xs_bf = sg4[:, sr, 0, c0 : c0 + 512]
                    xd_bf = dg4[:, q, 0, 512 * sr : 512 * sr + 512]

                    # ---- dot input (independent of MLP chain) ----
                    prod = wp.tile([128, 512], BF16, tag="prod")
                    nc.vector.tensor_tensor(
                        out=prod[:], in0=xs_bf, in1=xd_bf, op=ALU.mult
                    )

                    # ---- L2 (fp8 DoubleRow) ----
                    h2p = pp1.tile([128, 512], F32, tag="h2p")
                    nc.tensor.matmul(
                        out=h2p[:], lhsT=w2f[:], rhs=h1s4[:, g % 2, :, :],
                        perf_mode=DR, start=True, stop=True,
                    )
                    h2s = wp.tile([128, 512], BF16, tag="h2s")
                    nc.scalar.activation(out=h2s[:], in_=h2p[:], func=AF.Relu, bias=b2t[:])

                    # ---- L3 (rows 0:64) + gate L1 (rows 64:128), one PSUM ----
                    hg = pp1.tile([128, 512], F32, tag="hg")
                    nc.tensor.matmul(
                        out=hg[0:64, :], lhsT=w3t[:], rhs=h2s[:], start=True, stop=True
                    )
                    nc.tensor.matmul(
                        out=hg[64:128, :], lhsT=wg1s[:], rhs=xs_bf, start=True, stop=False
                    )
                    nc.tensor.matmul(
                        out=hg[64:128, :], lhsT=wg1dt[:], rhs=xd_bf, start=False, stop=True
                    )
                    hgs = wp.tile([128, 512], BF16, tag="hgs")
                    nc.vector.scalar_tensor_tensor(
                        out=hgs[:], in0=hg[:], scalar=b3g[:], in1=zeros[:],
                        op0=ALU.add, op1=ALU.max,
                    )

                    # ---- heads: rows (mlp, gd, dot) of one PSUM tile ----
                    hd = pp1.tile([3, 512], F32, tag="hd")
                    nc.tensor.matmul(
                        out=hd[:], lhsT=hw3[:], rhs=hgs[:], start=True, stop=False
                    )
                    nc.tensor.matmul(
                        out=hd[:], lhsT=ones3[:], rhs=prod[:], start=False, stop=True
                    )
                    hd_sb = wp.tile([3, 512], F32, tag="hd_sb")
                    nc.vector.tensor_copy(out=hd_sb[:], in_=hd[:])

                    # ---- back to edge-major: tt[:, 3j+k] = hd_sb[k, 128j+p] ----
                    for j in range(4):
                        nc.tensor.transpose(
                            out=tt_all[:, 12 * g + 3 * j : 12 * g + 3 * j + 3],
                            in_=hd_sb[:, 128 * j : 128 * (j + 1)],
                            identity=id3[:],
                        )

                # ---- batched tail over the whole macro (reads tt_all PSUM) ----
                mlp_v = tt_all[:].rearrange("p (c k) -> p k c", k=3)[:, 0, :]
                gd_v = tt_all[:].rearrange("p (c k) -> p k c", k=3)[:, 1, :]
                dot_p = tt_all[:].rearrange("p (c k) -> p k c", k=3)[:, 2, :]

                dot_v = wp.tile([128, 4 * GROUPS], F32, tag="dot_v")
                nc.vector.tensor_copy(out=dot_v[:], in_=dot_p)
                sig = wp.tile([128, 4 * GROUPS], F32, tag="sig")
                nc.scalar.activation(out=sig[:], in_=gd_v, func=AF.Sigmoid, bias=bg2dt[:])
                d1 = wp.tile([128, 4 * GROUPS], F32, tag="d1")
                nc.vector.scalar_tensor_tensor(
                    out=d1[:], in0=mlp_v, scalar=b4t[:], in1=dot_v[:],
                    op0=ALU.add, op1=ALU.subtract,
                )
                sd = wp.tile([128, 4 * GROUPS], F32, tag="sd")
                nc.vector.tensor_tensor(out=sd[:], in0=d1[:], in1=sig[:], op=ALU.mult)
                final_em = wp.tile([128, 4 * GROUPS], F32, tag="final_em")
                nc.vector.tensor_tensor(out=final_em[:], in0=sd[:], in1=dot_v[:], op=ALU.add)

                nc.sync.dma_start(
                    out=out[base : base + MACRO].rearrange("(p c) -> p c", c=MACRO // 128),
                    in_=final_em[:],
                )

    nc.compile()
    return nc


def _get_nc():
    if "nc" not in _CACHE:
        _CACHE["nc"] = build_nc()
    return _CACHE["nc"]


def _pack_idx16(vals, ncall, percall):
    """[nm, ncall*percall] int16 (slot order) -> [nm, 128, ncall*percall//16]
    gather-index layout: call k's idx t sits at (partition t%16,
    col k*(percall//16) + t//16), replicated across 16-partition groups."""
    nm = vals.shape[0]
    v = vals.reshape(nm, ncall, percall // 16, 16)
    arr = np.zeros((nm, 128, ncall * percall // 16), np.int16)
    arr[:, 0:16, :] = v.transpose(0, 3, 1, 2).reshape(nm, 16, -1)
    for k in range(1, 8):
        arr[:, 16 * k : 16 * (k + 1), :] = arr[:, 0:16, :]
    return arr


def _mixed_rows(tab):
    """[n, 128] f32 -> [n, 256] bf16-container mixed rows:
    u16 0..127 = bf16 feats, 128..191 = fp8e4m3 pairs, 192..255 = 0."""
    n = tab.shape[0]
    arr = np.zeros((n, ROW), dtype=BF)
    arr[:, 0:D] = tab.astype(BF)
    f8 = tab.astype(np.float32).astype(NF8).view(np.uint8).reshape(n, 64, 2)
    u16 = f8[:, :, 0].astype(np.uint16) | (f8[:, :, 1].astype(np.uint16) << 8)
    arr.view(np.uint16)[:, 128:192] = u16
    return arr


def _pack_w1(Wside):
    """[128 feats, 256 m] -> [128, 2, 256] fp8 DoubleRow lhsT:
    w[j, t, m] = W[2j+t, m] for j < 64, rows 64..127 zero."""
    w = np.zeros((128, 2, 256), dtype=NF8)
    w[0:64, 0, :] = Wside[0::2, :].astype(np.float32).astype(NF8)
    w[0:64, 1, :] = Wside[1::2, :].astype(np.float32).astype(NF8)
    return w


def kernel(h_user, h_item, src, dst,
           W1, b1, W2, b2, W3, b3, W4, b4,
           Wg1, bg1, Wg2, bg2, _trace=False):
    nc = _get_nc()

    src = np.asarray(src).astype(np.int64)
    dst = np.asarray(dst).astype(np.int64)

    # ---- host-side packing: (src_range, dst_range) bucket quotas ----
    sr = src // RANGE
    dr = dst // RANGE
    b = sr * NR + dr
    counts = np.bincount(b, minlength=16)
    cap = (NMAC_TOT // 2) * QUOTA
    assert counts.max() <= cap, counts
    order = np.argsort(b, kind="stable")
    slot_edge = np.full((NMAC_TOT, MACRO), -1, dtype=np.int64)
    # global macro ids serving dr pair 0/1 (even per-core idx) and 2/3 (odd)
    mac_even = np.array([c * NMACRO + j for c in range(N_CORES) for j in range(0, NMACRO, 2)])
    mac_odd = np.array([c * NMACRO + j for c in range(N_CORES) for j in range(1, NMACRO, 2)])
    pos = 0
    for bb in range(16):
        nb = counts[bb]
        arr = np.full(cap, -1, dtype=np.int64)
        arr[:nb] = order[pos : pos + nb]
        pos += nb
        sb, db_ = bb // NR, bb % NR
        macs = mac_even if db_ < 2 else mac_odd
        col0 = 1024 * sb + 512 * (db_ % 2)
        slot_edge[macs, col0 : col0 + 512] = arr.reshape(len(macs), 512)

    valid = slot_edge >= 0
    e_clip = np.clip(slot_edge, 0, None)
    colr = (np.arange(MACRO) // 1024)[None, :]
    s16 = np.where(valid, src[e_clip] - RANGE * colr, 0).astype(np.int16)

    # dg col j <-> sg col: q = j//2048, sr = (j%2048)//512, k = j%512
    j = np.arange(MACRO)
    sgcol = 1024 * ((j % 2048) // 512) + 512 * (j // 2048) + (j % 512)
    slot_edge_dg = slot_edge[:, sgcol]
    valid_dg = slot_edge_dg >= 0
    drb_m = (np.arange(NMAC_TOT) % 2 * 2)[:, None]
    d16 = np.where(
        valid_dg,
        dst[np.clip(slot_edge_dg, 0, None)] - RANGE * (drb_m + j[None, :] // 2048),
        0,
    ).astype(np.int16)

    srcc = _pack_idx16(s16, NR, 1024)
    dstc = _pack_idx16(d16, 2, 2048)

    # ---- weights / constants: pack the byte blob ----
    W1 = np.asarray(W1, dtype=np.float32)
    W2 = np.asarray(W2, dtype=np.float32)
    w2f = np.zeros((128, 2, 128), dtype=NF8)
    w2f[:, 0, :] = W2[0:128].astype(NF8)
    w2f[:, 1, :] = W2[128:256].astype(NF8)
    hw3 = np.zeros((128, 4), np.float32)
    hw3[0:64, 0] = np.asarray(W4)[:, 0]
    hw3[64:128, 1] = np.asarray(Wg2)[:, 1] - np.asarray(Wg2)[:, 0]
    ones3 = np.zeros((128, 4), np.float32)
    ones3[:, 2] = 1.0
    b3g = np.concatenate([np.asarray(b3), np.asarray(bg1)]).astype(np.float32)

    def as_u8(a):
        return np.ascontiguousarray(a).view(np.uint8).reshape(128, -1)

    id3b = np.zeros((128, 12), np.uint8)
    id3b[0:3] = np.eye(3, dtype=np.float32).view(np.uint8).reshape(3, 12)
    parts = {
        "w1sf": as_u8(_pack_w1(W1[0:128, :])),
        "w1df": as_u8(_pack_w1(W1[128:256, :])),
        "w2f": as_u8(w2f),
        "w3": as_u8(np.asarray(W3, dtype=np.float32).astype(BF)),
        "wg1s": as_u8(np.asarray(Wg1, dtype=np.float32)[0:128, :].astype(BF)),
        "wg1d": as_u8(np.asarray(Wg1, dtype=np.float32)[128:256, :].astype(BF)),
        "hw3": as_u8(hw3.astype(BF)),
        "ones3": as_u8(ones3.astype(BF)),
        "b1a": as_u8(np.asarray(b1, np.float32)[0:128].reshape(128, 1)),
        "b1b": as_u8(np.asarray(b1, np.float32)[128:256].reshape(128, 1)),
        "b2t": as_u8(np.asarray(b2, np.float32).reshape(128, 1)),
        "b3g": as_u8(b3g.reshape(128, 1)),
        "b4t": as_u8(np.full((128, 1), np.asarray(b4, np.float32)[0], np.float32)),
        "bg2dt": as_u8(np.full((128, 1), np.float32(bg2[1] - bg2[0]), np.float32)),
        "id3": id3b,
        "zeros": np.zeros((128, 1024), np.uint8),
    }
    cblob = np.zeros((128, CBYTES), np.uint8)
    for name, _sz in _OFFS:
        cblob[:, OFF[name] : OFF[name] + _sz] = parts[name]

    common = {
        "hum": _mixed_rows(np.asarray(h_user, dtype=np.float32)),
        "him": _mixed_rows(np.asarray(h_item, dtype=np.float32)),
        "cblob": cblob,
    }
    in_maps = []
    for c in range(N_CORES):
        mm = dict(common)
        mm["srcc"] = srcc[c * NMACRO : (c + 1) * NMACRO]
        mm["dstc"] = dstc[c * NMACRO : (c + 1) * NMACRO]
        in_maps.append(mm)

    res = run_bass_kernel_spmd(
        nc, in_maps, core_ids=list(range(N_CORES)), trace=_trace
    )
    outs = np.concatenate([res.results[c]["out"] for c in range(N_CORES)])
    # out linear index = p*32 + c; value at (m, p, c) belongs to sg slot 128c+p
    vals = outs.reshape(NMAC_TOT, 128, MACRO // 128)
    cols = np.arange(MACRO // 128)[None, :] * 128 + np.arange(128)[:, None]
    se = slot_edge[:, cols]          # [m, p, c] -> edge id
    mask = se >= 0
    final = np.zeros(N_EDGES, dtype=np.float32)
    final[se[mask]] = vals[mask]
    if _trace:
        kernel._last_result = res
    return final


kernel._last_result = None


# revision 41
# speedup vs baseline: 1.0989x; 1.0989x over previous
"""Trainium2 Bass kernel for nn_MixedPredictor (gnn_message_passing).

final[e] = softmax(gates)[0] * dot(h_user[src[e]], h_item[dst[e]])
         + softmax(gates)[1] * MLP(concat(h_user[src[e]], h_item[dst[e]]))

Strategy (8 NeuronCores, data-parallel over edges):
  - Edge rows are fetched with InstDMAGatherAnt (nc.gpsimd.dma_gather):
    one instruction gathers 1024-2048 rows (994 ns fixed + 0.34 ns/desc on
    the Pool engine) vs indirect_dma_start's 128 rows per ~1 us call. Node
    tables are repacked host-side into 512-byte mixed rows
    [bf16 f0..f127 | fp8e4m3 pair-packed f0..f127 | pad] so transpose=True
    lands each gather FEATURE-MAJOR in SBUF (no PE transposes, no
    PSUM->SBUF copies) with both a bf16 plane (dot product, gate layer) and
    an fp8 plane (MLP layers 1-2 run as DoubleRow matmuls at 2x PE rate).
    512 B descriptors ride the DMA latency floor, so the fp8 plane is free.
  - dma_gather indices are int16, so the 100k-row tables are split in 4
    ranges of 25000 rows. Edges are bucketed host-side by (src_range,
    dst_range); each 4096-edge macro serves one PAIR of dst ranges
    (even macros dr 0/1, odd macros dr 2/3) with a 512-slot quota per
    bucket, giving 4 src gathers (1024 idxs) + 2 dst gathers (2048) per
    macro, each writing one contiguous span.
  - Layer 3 and the gate layer share one PSUM tile ([0:64)/[64:128)
    partitions) -> one relu covers both. The heads (mlp, gate-delta, dot)
    are three rows of one PSUM tile via two accumulating matmuls, PE-
    transposed to edge-major; the final combine
    final = dot + sigmoid(gd)*(mlp + b4 - dot) is batched once per macro.
  - Elementwise work is split so each engine's in-order queue only holds
    early-chain ops: Activation does the L1/L2 relus + sigmoid, DVE does the
    L3/gate relu, dot multiply, head copies and the batched tail. Gathers
    prefetch 3 macros ahead, so Pool descriptor generation never stalls
    compute.
"""

import numpy as np
import ml_dtypes

import concourse.bass as bass
import concourse.bacc as bacc
import concourse.mybir as mybir
import concourse.tile as tile
from concourse.bass_utils import run_bass_kernel_spmd

N_CORES = 8
N_USERS = 100000
N_ITEMS = 100000
N_EDGES = 500000
D = 128

MACRO = 4096            # edges per macro tile
NMACRO = 16             # macros per core (even: dr 0/1, odd: dr 2/3)
GROUPS = 8              # 512-edge groups per macro
NMAC_TOT = N_CORES * NMACRO      # 128
E_CORE = NMACRO * MACRO          # 65536
E_PAD = N_CORES * E_CORE         # 524288

RANGE = 25000           # rows per gather-index range (int16-safe)
NR = 4                  # index ranges covering 100k rows
QUOTA = 512             # slots per (sr, dr) bucket per eligible macro
ROW = 256               # mixed-row u16 elements: 128 bf16 + 64 fp8-pair + 64 pad

F32 = mybir.dt.float32
BF16 = mybir.dt.bfloat16
F8 = mybir.dt.float8e4
I16 = mybir.dt.int16
AF = mybir.ActivationFunctionType
ALU = mybir.AluOpType
DR = mybir.MatmulPerfMode.DoubleRow
BF = ml_dtypes.bfloat16
NF8 = ml_dtypes.float8_e4m3

_OFFS = [
    ("w1sf", 512), ("w1df", 512), ("w2f", 256), ("w3", 128),
    ("wg1s", 128), ("wg1d", 128), ("hw3", 8), ("ones3", 8),
    ("b1a", 4), ("b1b", 4), ("b2t", 4), ("b3g", 4), ("b4t", 4),
    ("bg2dt", 4), ("id3", 12), ("zeros", 1024),
]
OFF = {}
_o = 0
for _n, _sz in _OFFS:
    OFF[_n] = _o
    _o += _sz
CBYTES = _o

_CACHE = {}


def build_nc(nmacro=NMACRO):
    nc = bacc.Bacc(
        "TRN2",
        target_bir_lowering=False,
        debug=False,
        enable_asserts=False,
        num_devices=N_CORES,
    )

    hu = nc.dram_tensor("hum", [N_USERS, ROW], BF16, kind="ExternalInput").ap()
    hi = nc.dram_tensor("him", [N_ITEMS, ROW], BF16, kind="ExternalInput").ap()
    srcs = nc.dram_tensor("srcc", [NMACRO, 128, 256], I16, kind="ExternalInput").ap()
    dsts = nc.dram_tensor("dstc", [NMACRO, 128, 256], I16, kind="ExternalInput").ap()
    U8 = mybir.dt.uint8
    cbd = nc.dram_tensor("cblob", [128, CBYTES], U8, kind="ExternalInput").ap()

    out = nc.dram_tensor("out", [nmacro * MACRO], F32, kind="ExternalOutput").ap()

    with tile.TileContext(nc) as tc:
        with (
            tc.tile_pool(name="const", bufs=1) as cp,
            tc.tile_pool(name="gather", bufs=3) as gp,
            tc.tile_pool(name="work", bufs=4) as wp,
            tc.tile_pool(name="psum1", bufs=1, space="PSUM") as pp1,
            tc.tile_pool(name="psum2", bufs=1, space="PSUM") as pp2,
        ):
            # ---- constants: one packed byte-blob DMA, bitcast views ----
            cb = cp.tile([128, CBYTES], mybir.dt.uint8, tag="cb")
            nc.sync.dma_start(out=cb[:], in_=cbd[:, :])

            def cview(off, nbytes, dt_):
                return cb[:, off : off + nbytes].bitcast(dt_)

            w1sf = cview(OFF["w1sf"], 512, F8).rearrange("p (two m) -> p two m", two=2)
            w1df = cview(OFF["w1df"], 512, F8).rearrange("p (two m) -> p two m", two=2)
            w2f = cview(OFF["w2f"], 256, F8).rearrange("p (two m) -> p two m", two=2)
            w3t = cview(OFF["w3"], 128, BF16)
            wg1s = cview(OFF["wg1s"], 128, BF16)
            wg1dt = cview(OFF["wg1d"], 128, BF16)
            hw3 = cview(OFF["hw3"], 8, BF16)[:, 0:3]
            ones3 = cview(OFF["ones3"], 8, BF16)[:, 0:3]
            b1a = cview(OFF["b1a"], 4, F32)
            b1b = cview(OFF["b1b"], 4, F32)
            b2t = cview(OFF["b2t"], 4, F32)
            b3g = cview(OFF["b3g"], 4, F32)
            b4t = cview(OFF["b4t"], 4, F32)
            bg2dt = cview(OFF["bg2dt"], 4, F32)
            id3 = cb[0:3, OFF["id3"] : OFF["id3"] + 12].bitcast(F32)
            zeros = cview(OFF["zeros"], 1024, BF16)

            for m in range(nmacro):
                base = m * MACRO
                drb = 0 if m % 2 == 0 else 2   # dst ranges served by this macro

                idx_s = gp.tile([128, 256], I16, tag="idx_s")
                nc.sync.dma_start(out=idx_s[:], in_=srcs[m, :, :])
                idx_d = gp.tile([128, 256], I16, tag="idx_d")
                nc.sync.dma_start(out=idx_d[:], in_=dsts[m, :, :])

                # sg4[p, r, h, e]: src range r, u16-half h (0=bf16, 1=fp8), col e
                sg4 = gp.tile([128, NR, 2, 1024], BF16, tag="sg4")
                dg4 = gp.tile([128, 2, 2, 2048], BF16, tag="dg4")
                def sgather(r):
                    nc.gpsimd.dma_gather(
                        out_ap=sg4[:, r, :, :],
                        in_ap=hu[RANGE * r : RANGE * (r + 1), :],
                        idxs_ap=idx_s[:, 64 * r : 64 * (r + 1)],
                        num_idxs=1024,
                        num_idxs_reg=1024,
                        elem_size=ROW,
                        transpose=True,
                        single_packet=False,
                    )
                def dgather(q):
                    nc.gpsimd.dma_gather(
                        out_ap=dg4[:, q, :, :],
                        in_ap=hi[RANGE * (drb + q) : RANGE * (drb + q + 1), :],
                        idxs_ap=idx_d[:, 128 * q : 128 * (q + 1)],
                        num_idxs=2048,
                        num_idxs_reg=2048,
                        elem_size=ROW,
                        transpose=True,
                        single_packet=False,
                    )
                # src range 0 + both dst ranges first: groups 0-1 unblock
                # after 3 calls instead of 6
                sgather(0)
                dgather(0)
                dgather(1)
                for r in range(1, NR):
                    sgather(r)

                sg_f8 = sg4[:].bitcast(F8).rearrange(
                    "p r h (e two) -> p r h two e", two=2
                )
                dg_f8 = dg4[:].bitcast(F8).rearrange(
                    "p q h (e two) -> p q h two e", two=2
                )

                tt_all = pp1.tile([128, 12 * GROUPS], F32, tag="tt")

                for gpair in range(GROUPS // 2):
                  # ---- L1 (fp8 DoubleRow) for a PAIR of groups into one
                  # [128, 1024] PSUM so the relus run 1024 wide (halved init)
                  h1a2 = pp2.tile([128, 1024], F32, tag="h1a2")
                  h1b2 = pp2.tile([128, 1024], F32, tag="h1b2")
                  for g in (2 * gpair, 2 * gpair + 1):
                    sr, q = g // 2, g % 2
                    c0 = 512 * q
                    xs_f8 = sg_f8[:, sr, 1, :, c0 : c0 + 512]
                    xd_f8 = dg_f8[:, q, 1, :, 512 * sr : 512 * sr + 512]
                    csl = slice(512 * (g % 2), 512 * (g % 2) + 512)
                    for msl, h1 in ((slice(0, 128), h1a2), (slice(128, 256), h1b2)):
                        nc.tensor.matmul(
                            out=h1[:, csl], lhsT=w1sf[:, :, msl], rhs=xs_f8,
                            perf_mode=DR, start=True, stop=False,
                        )
                        nc.tensor.matmul(
                            out=h1[:, csl], lhsT=w1df[:, :, msl], rhs=xd_f8,
                            perf_mode=DR, start=False, stop=True,
                        )
                  # h1s4[p, pair, plane, e]: relu'd h1 in fp8, both groups
                  h1s4 = wp.tile([128, 2, 2, 512], F8, tag="h1s4")
                  nc.scalar.activation(
                      out=h1s4[:, :, 0, :], in_=h1a2[:], func=AF.Relu, bias=b1a[:],
                  )
                  nc.scalar.activation(
                      out=h1s4[:, :, 1, :], in_=h1b2[:], func=AF.Relu, bias=b1b[:],
                  )

                  for g in (2 * gpair, 2 * gpair + 1):
                    sr, q = g // 2, g % 2
                    c0 = 512 * q
                    xs_bf = sg4[:, sr, 0, c0 : c0 + 512]
                    xd_bf = dg4[:, q, 0, 512 * sr : 512 * sr + 512]

                    # ---- dot input (independent of MLP chain) ----
                    prod = wp.tile([128, 512], BF16, tag="prod")
                    nc.vector.tensor_tensor(
                        out=prod[:], in0=xs_bf, in1=xd_bf, op=ALU.mult
                    )

                    # ---- gate L1 first: depends only on gathers, fills the
                    # PE stall while the L1 relu finishes ----
                    hg = pp1.tile([128, 512], F32, tag="hg")
                    nc.tensor.matmul(
                        out=hg[64:128, :], lhsT=wg1s[:], rhs=xs_bf, start=True, stop=False
                    )
                    nc.tensor.matmul(
                        out=hg[64:128, :], lhsT=wg1dt[:], rhs=xd_bf, start=False, stop=True
                    )

                    # ---- L2 (fp8 DoubleRow) ----
                    h2p = pp1.tile([128, 512], F32, tag="h2p")
                    nc.tensor.matmul(
                        out=h2p[:], lhsT=w2f[:], rhs=h1s4[:, g % 2, :, :],
                        perf_mode=DR, start=True, stop=True,
                    )
                    h2s = wp.tile([128, 512], BF16, tag="h2s")
                    nc.scalar.activation(out=h2s[:], in_=h2p[:], func=AF.Relu, bias=b2t[:])

                    # ---- L3 into rows 0:64 of the shared tile ----
                    nc.tensor.matmul(
                        out=hg[0:64, :], lhsT=w3t[:], rhs=h2s[:], start=True, stop=True
                    )
                    hgs = wp.tile([128, 512], BF16, tag="hgs")
                    nc.vector.scalar_tensor_tensor(
                        out=hgs[:], in0=hg[:], scalar=b3g[:], in1=zeros[:],
                        op0=ALU.add, op1=ALU.max,
                    )

                    # ---- heads: rows (mlp, gd, dot) of one PSUM tile ----
                    hd = pp1.tile([3, 512], F32, tag="hd")
                    nc.tensor.matmul(
                        out=hd[:], lhsT=ones3[:], rhs=prod[:], start=True, stop=False
                    )
                    nc.tensor.matmul(
                        out=hd[:], lhsT=hw3[:], rhs=hgs[:], start=False, stop=True
                    )
                    hd_sb = wp.tile([3, 512], F32, tag="hd_sb")
                    nc.vector.tensor_copy(out=hd_sb[:], in_=hd[:])

                    # ---- back to edge-major: tt[:, 3j+k] = hd_sb[k, 128j+p] ----
                    for j in range(4):
                        nc.tensor.transpose(
                            out=tt_all[:, 12 * g + 3 * j : 12 * g + 3 * j + 3],
                            in_=hd_sb[:, 128 * j : 128 * (j + 1)],
                            identity=id3[:],
                        )

                # ---- batched tail over the whole macro (reads tt_all PSUM) ----
                mlp_v = tt_all[:].rearrange("p (c k) -> p k c", k=3)[:, 0, :]
                gd_v = tt_all[:].rearrange("p (c k) -> p k c", k=3)[:, 1, :]
                dot_p = tt_all[:].rearrange("p (c k) -> p k c", k=3)[:, 2, :]

                dot_v = wp.tile([128, 4 * GROUPS], F32, tag="dot_v")
                nc.vector.tensor_copy(out=dot_v[:], in_=dot_p)
                sig = wp.tile([128, 4 * GROUPS], F32, tag="sig")
                nc.scalar.activation(out=sig[:], in_=gd_v, func=AF.Sigmoid, bias=bg2dt[:])
                d1 = wp.tile([128, 4 * GROUPS], F32, tag="d1")
                nc.vector.scalar_tensor_tensor(
                    out=d1[:], in0=mlp_v, scalar=b4t[:], in1=dot_v[:],
                    op0=ALU.add, op1=ALU.subtract,
                )
                sd = wp.tile([128, 4 * GROUPS], F32, tag="sd")
                nc.vector.tensor_tensor(out=sd[:], in0=d1[:], in1=sig[:], op=ALU.mult)
                final_em = wp.tile([128, 4 * GROUPS], F32, tag="final_em")
                nc.vector.tensor_tensor(out=final_em[:], in0=sd[:], in1=dot_v[:], op=ALU.add)

                nc.sync.dma_start(
                    out=out[base : base + MACRO].rearrange("(p c) -> p c", c=MACRO // 128),
                    in_=final_em[:],
                )

    nc.compile()
    return nc


def _get_nc():
    if "nc" not in _CACHE:
        _CACHE["nc"] = build_nc()
    return _CACHE["nc"]


def _pack_idx16(vals, ncall, percall):
    """[nm, ncall*percall] int16 (slot order) -> [nm, 128, ncall*percall//16]
    gather-index layout: call k's idx t sits at (partition t%16,
    col k*(percall//16) + t//16), replicated across 16-partition groups."""
    nm = vals.shape[0]
    v = vals.reshape(nm, ncall, percall // 16, 16)
    arr = np.zeros((nm, 128, ncall * percall // 16), np.int16)
    arr[:, 0:16, :] = v.transpose(0, 3, 1, 2).reshape(nm, 16, -1)
    for k in range(1, 8):
        arr[:, 16 * k : 16 * (k + 1), :] = arr[:, 0:16, :]
    return arr


def _mixed_rows(tab):
    """[n, 128] f32 -> [n, 256] bf16-container mixed rows:
    u16 0..127 = bf16 feats, 128..191 = fp8e4m3 pairs, 192..255 = 0."""
    n = tab.shape[0]
    arr = np.zeros((n, ROW), dtype=BF)
    arr[:, 0:D] = tab.astype(BF)
    f8 = tab.astype(np.float32).astype(NF8).view(np.uint8).reshape(n, 64, 2)
    u16 = f8[:, :, 0].astype(np.uint16) | (f8[:, :, 1].astype(np.uint16) << 8)
    arr.view(np.uint16)[:, 128:192] = u16
    return arr


def _pack_w1(Wside):
    """[128 feats, 256 m] -> [128, 2, 256] fp8 DoubleRow lhsT:
    w[j, t, m] = W[2j+t, m] for j < 64, rows 64..127 zero."""
    w = np.zeros((128, 2, 256), dtype=NF8)
    w[0:64, 0, :] = Wside[0::2, :].astype(np.float32).astype(NF8)
    w[0:64, 1, :] = Wside[1::2, :].astype(np.float32).astype(NF8)
    return w


def kernel(h_user, h_item, src, dst,
           W1, b1, W2, b2, W3, b3, W4, b4,
           Wg1, bg1, Wg2, bg2, _trace=False):
    nc = _get_nc()

    src = np.asarray(src).astype(np.int64)
    dst = np.asarray(dst).astype(np.int64)

    # ---- host-side packing: (src_range, dst_range) bucket quotas ----
    sr = src // RANGE
    dr = dst // RANGE
    b = sr * NR + dr
    counts = np.bincount(b, minlength=16)
    cap = (NMAC_TOT // 2) * QUOTA
    assert counts.max() <= cap, counts
    order = np.argsort(b, kind="stable")
    slot_edge = np.full((NMAC_TOT, MACRO), -1, dtype=np.int64)
    # global macro ids serving dr pair 0/1 (even per-core idx) and 2/3 (odd)
    mac_even = np.array([c * NMACRO + j for c in range(N_CORES) for j in range(0, NMACRO, 2)])
    mac_odd = np.array([c * NMACRO + j for c in range(N_CORES) for j in range(1, NMACRO, 2)])
    pos = 0
    for bb in range(16):
        nb = counts[bb]
        arr = np.full(cap, -1, dtype=np.int64)
        arr[:nb] = order[pos : pos + nb]
        pos += nb
        sb, db_ = bb // NR, bb % NR
        macs = mac_even if db_ < 2 else mac_odd
        col0 = 1024 * sb + 512 * (db_ % 2)
        slot_edge[macs, col0 : col0 + 512] = arr.reshape(len(macs), 512)

    valid = slot_edge >= 0
    e_clip = np.clip(slot_edge, 0, None)
    colr = (np.arange(MACRO) // 1024)[None, :]
    s16 = np.where(valid, src[e_clip] - RANGE * colr, 0).astype(np.int16)

    # dg col j <-> sg col: q = j//2048, sr = (j%2048)//512, k = j%512
    j = np.arange(MACRO)
    sgcol = 1024 * ((j % 2048) // 512) + 512 * (j // 2048) + (j % 512)
    slot_edge_dg = slot_edge[:, sgcol]
    valid_dg = slot_edge_dg >= 0
    drb_m = (np.arange(NMAC_TOT) % 2 * 2)[:, None]
    d16 = np.where(
        valid_dg,
        dst[np.clip(slot_edge_dg, 0, None)] - RANGE * (drb_m + j[None, :] // 2048),
        0,
    ).astype(np.int16)

    srcc = _pack_idx16(s16, NR, 1024)
    dstc = _pack_idx16(d16, 2, 2048)

    # ---- weights / constants: pack the byte blob ----
    W1 = np.asarray(W1, dtype=np.float32)
    W2 = np.asarray(W2, dtype=np.float32)
    w2f = np.zeros((128, 2, 128), dtype=NF8)
    w2f[:, 0, :] = W2[0:128].astype(NF8)
    w2f[:, 1, :] = W2[128:256].astype(NF8)
    hw3 = np.zeros((128, 4), np.float32)
    hw3[0:64, 0] = np.asarray(W4)[:, 0]
    hw3[64:128, 1] = np.asarray(Wg2)[:, 1] - np.asarray(Wg2)[:, 0]
    ones3 = np.zeros((128, 4), np.float32)
    ones3[:, 2] = 1.0
    b3g = np.concatenate([np.asarray(b3), np.asarray(bg1)]).astype(np.float32)

    def as_u8(a):
        return np.ascontiguousarray(a).view(np.uint8).reshape(128, -1)

    id3b = np.zeros((128, 12), np.uint8)
    id3b[0:3] = np.eye(3, dtype=np.float32).view(np.uint8).reshape(3, 12)
    parts = {
        "w1sf": as_u8(_pack_w1(W1[0:128, :])),
        "w1df": as_u8(_pack_w1(W1[128:256, :])),
        "w2f": as_u8(w2f),
        "w3": as_u8(np.asarray(W3, dtype=np.float32).astype(BF)),
        "wg1s": as_u8(np.asarray(Wg1, dtype=np.float32)[0:128, :].astype(BF)),
        "wg1d": as_u8(np.asarray(Wg1, dtype=np.float32)[128:256, :].astype(BF)),
        "hw3": as_u8(hw3.astype(BF)),
        "ones3": as_u8(ones3.astype(BF)),
        "b1a": as_u8(np.asarray(b1, np.float32)[0:128].reshape(128, 1)),
        "b1b": as_u8(np.asarray(b1, np.float32)[128:256].reshape(128, 1)),
        "b2t": as_u8(np.asarray(b2, np.float32).reshape(128, 1)),
        "b3g": as_u8(b3g.reshape(128, 1)),
        "b4t": as_u8(np.full((128, 1), np.asarray(b4, np.float32)[0], np.float32)),
        "bg2dt": as_u8(np.full((128, 1), np.float32(bg2[1] - bg2[0]), np.float32)),
        "id3": id3b,
        "zeros": np.zeros((128, 1024), np.uint8),
    }
    cblob = np.zeros((128, CBYTES), np.uint8)
    for name, _sz in _OFFS:
        cblob[:, OFF[name] : OFF[name] + _sz] = parts[name]

    common = {
        "hum": _mixed_rows(np.asarray(h_user, dtype=np.float32)),
        "him": _mixed_rows(np.asarray(h_item, dtype=np.float32)),
        "cblob": cblob,
    }
    in_maps = []
    for c in range(N_CORES):
        mm = dict(common)
        mm["srcc"] = srcc[c * NMACRO : (c + 1) * NMACRO]
        mm["dstc"] = dstc[c * NMACRO : (c + 1) * NMACRO]
        in_maps.append(mm)

    res = run_bass_kernel_spmd(
        nc, in_maps, core_ids=list(range(N_CORES)), trace=_trace
    )
    outs = np.concatenate([res.results[c]["out"] for c in range(N_CORES)])
    # out linear index = p*32 + c; value at (m, p, c) belongs to sg slot 128c+p
    vals = outs.reshape(NMAC_TOT, 128, MACRO // 128)
    cols = np.arange(MACRO // 128)[None, :] * 128 + np.arange(128)[:, None]
    se = slot_edge[:, cols]          # [m, p, c] -> edge id
    mask = se >= 0
    final = np.zeros(N_EDGES, dtype=np.float32)
    final[se[mask]] = vals[mask]
    if _trace:
        kernel._last_result = res
    return final


kernel._last_result = None
